# revision 2
# baseline (speedup 1.0000x reference)
"""Trainium2 Bass kernel for DONNSentimentClassifier — composed-map version.

Per token, the 20 Euler substeps with constant forcing g compose (exactly, to
O(g^2) ~ 1e-6 rel) into  w' = F1(u) w + F2(u) g + F3p(u) w^2 conj(g),
u = |w|^2, with F1/F2/F3p smooth per-oscillator functions of u fitted as
degree-DEG polynomials in xi = a*u + b (per-oscillator coefficients live in
the scalar_tensor_tensor per-partition scalar slot).

Single fused pass: layer-1 and layer-2 states share one [64, 128] tile
(columns = (comp r|i, layer, batch)); layer 2 lags 2 tokens, with its state
initialized at the double-preimage of w0 so the first two (zero-forcing)
waves land it exactly on w0 when its token 0 begins.  g2 is produced on
PE+ACT from layer-1's output one wave ahead; the classifier head runs on
PE+ACT in parallel with the next wave's DVE work.  8-way batch parallel.
"""

import sys

for p in ("/opt/trn_rl_repo", "/root/.axon_site/_ro/trn_rl_repo"):
    if p not in sys.path:
        sys.path.append(p)

import numpy as np

import concourse.bass as bass
import concourse.mybir as mybir
from concourse.bass_utils import run_bass_kernel_spmd
from concourse.tile import TileContext

F32 = mybir.dt.float32
AF = mybir.ActivationFunctionType
OP = mybir.AluOpType

B, T, U, ED, V, PD, NC_OUT = 256, 512, 64, 100, 32000, 20, 2
NUM_STEPS = 20
DT = 1e-3
SCALE = 0.2
MU = 1.0
C0 = 1.0 + DT * MU
SQDT = float(np.sqrt(DT))
GS = float(np.float32(DT * np.sqrt(DT) * SCALE))
N_CORES = 8
W = B // N_CORES  # 32 batch rows per core
LW = 2 * W  # (layer, batch) width = 64
CW = 2 * LW  # full complex tile width = 128

DEG = 4
USE_F3 = False
NF = 6 if USE_F3 else 4
# engine offloads for the map (all sim-tuned)
S_ON_ACT = False      # S = w*w on ACT (Square)
U_ON_POOL = True      # u = Sr+Si on GPSIMD
STARTS_ON_ACT = False # Horner chain starts a1 = xi*s1 on ACT
POOL_HORNER = 0       # HW: GPSIMD cannot run scalar_tensor_tensor
POOL_PRODUCTS = 4     # how many combine products run on GPSIMD (plain TT)
# fit domain for u = |w|^2 (w-units)
U_LO, U_HI = 1e-7, 1.35e-3
XI_A = 2.0 / (U_HI - U_LO)
XI_B = -1.0 - 2.0 * U_LO / (U_HI - U_LO)

CHUNK = 16  # waves per For_i body; ring slots
XI_ON_ACT = False
C0_ON_ACT = False

_CACHE = {}


def _split_waits(nc, cap=1):
    """This walrus build allows ~1 sync-wait per instruction; move excess
    waits onto single-wait NoOps (ported from the baseline kernel)."""
    nop_id = [0]
    for f in nc.m.functions:
        for bb in f.blocks:
            il = bb.instructions
            pos = 0
            while pos < len(il):
                ins = il[pos]
                si = ins.sync_info
                if si is None or si.on_wait is None or len(si.on_wait) <= cap:
                    pos += 1
                    continue
                waits = list(si.on_wait)
                keep, extra = waits[-cap:], waits[:-cap]
                for w in extra:
                    nop = mybir.InstNoOp(
                        name=f"waitnop_{nop_id[0]}", ins=[], outs=[]
                    )
                    nop_id[0] += 1
                    nop.engine = ins.engine
                    nop.sync_info = mybir.SyncInfo(on_wait=[w], on_update=[])
                    il.insert(pos, nop)
                    pos += 1
                ins.sync_info = mybir.SyncInfo(
                    on_wait=keep, on_update=list(si.on_update or [])
                )
                pos += 1


# ----------------------------------------------------------------- fitting
def _compose_F(u0, beta):
    """u0 [G] float64, beta scalar (rad/s). Returns F1, F2, F3 complex128."""
    u = u0.copy()
    w = np.sqrt(u0).astype(np.complex128)
    F1 = np.ones_like(w)
    LA = np.zeros_like(w)
    LB = np.zeros_like(w)
    for _ in range(NUM_STEPS):
        s = C0 - u + 1j * beta * DT
        p = s - u
        q = -w * w
        LA, LB = p * LA + q * np.conj(LB), p * LB + q * np.conj(LA)
        LA = LA + 1.0
        w = s * w
        F1 = F1 * s
        u = (w * np.conj(w)).real
    return F1, LA, LB


def _fit_coeffs(beta_vec, deg=DEG, npts=4000):
    """Returns cf [U, NF, deg+1] float32 in device chain order
    (s_1..s_deg, c0) for xi-power Horner via (acc+s)*xi, and the layer-2
    double-preimage init (w_pre complex128 [U])."""
    xi = np.cos(np.linspace(0, np.pi, npts))
    u = (xi - XI_B) / XI_A
    Vm = np.polynomial.polynomial.polyvander(xi, deg)
    cf = np.zeros((U, NF, deg + 1), np.float64)
    wpre = np.zeros(U, np.complex128)
    u0 = (0.1 * SQDT) ** 2
    for j, bj in enumerate(beta_vec):
        F1, F2, F3 = _compose_F(u, bj)
        funcs = [F1.real, F1.imag, F2.real, F2.imag]
        if USE_F3:
            F3p = F3 / u
            funcs += [F3p.real, F3p.imag]
        for k, f in enumerate(funcs):
            c, *_ = np.linalg.lstsq(Vm, f, rcond=None)
            # chain order: s_k = c_{deg+1-k} for k=1..deg, then c0
            cf[j, k, :deg] = c[deg:0:-1]
            cf[j, k, deg] = c[0]
        # invert the unforced radial map twice: find u2 with
        # u1 = u2*|F1(u2)|^2, u0 = u1*|F1(u1)|^2
        def fwd(uv):
            F1v, _, _ = _compose_F(np.array([uv]), bj)
            return uv * abs(F1v[0]) ** 2, F1v[0]

        lo, hi = 1e-8, u0
        for _ in range(80):
            mid = 0.5 * (lo + hi)
            u1m, _ = fwd(mid)
            u0m, _ = fwd(u1m)
            if u0m < u0:
                lo = mid
            else:
                hi = mid
        u2 = 0.5 * (lo + hi)
        u1, F1a = fwd(u2)
        _, F1b = fwd(u1)
        # w0 = F1(u1)*F1(u2)*w_pre with w0 = 0.1*sqrt(DT) real
        wpre[j] = (0.1 * SQDT) / (F1a * F1b)
    return cf.astype(np.float32), wpre


# ------------------------------------------------------------------ build
def _build(repeat=1):
    key = ("nc", repeat)
    if key in _CACHE:
        return _CACHE[key]
    nc = bass.Bass()

    NCO = NF * (DEG + 1)
    g1 = nc.declare_dram_parameter("g1", [U, (T + CHUNK) * 2 * W], F32, isOutput=False)
    cf = nc.declare_dram_parameter("cf", [U, NCO], F32, isOutput=False)
    cfn = nc.declare_dram_parameter("cfn", [U, NCO], F32, isOutput=False)
    winit = nc.declare_dram_parameter("winit", [U, CW], F32, isOutput=False)
    wp1a = nc.declare_dram_parameter("wp1a", [U, U], F32, isOutput=False)
    wp1b = nc.declare_dram_parameter("wp1b", [U, U], F32, isOutput=False)
    bp1 = nc.declare_dram_parameter("bp1", [U, 1], F32, isOutput=False)
    w2r = nc.declare_dram_parameter("w2r", [U, U], F32, isOutput=False)
    w2i = nc.declare_dram_parameter("w2i", [U, U], F32, isOutput=False)
    b2r = nc.declare_dram_parameter("b2r", [U, 1], F32, isOutput=False)
    b2i = nc.declare_dram_parameter("b2i", [U, 1], F32, isOutput=False)
    wp2a = nc.declare_dram_parameter("wp2a", [U, U], F32, isOutput=False)
    wp2b = nc.declare_dram_parameter("wp2b", [U, U], F32, isOutput=False)
    bp2 = nc.declare_dram_parameter("bp2", [U, 1], F32, isOutput=False)
    wpr = nc.declare_dram_parameter("wpr", [U, PD], F32, isOutput=False)
    bpr = nc.declare_dram_parameter("bpr", [PD, 1], F32, isOutput=False)
    wh = nc.declare_dram_parameter("wh", [PD, NC_OUT], F32, isOutput=False)
    bh = nc.declare_dram_parameter("bh", [NC_OUT, 1], F32, isOutput=False)
    out = nc.declare_dram_parameter("out", [NC_OUT, (T + 2) * W], F32, isOutput=True)

    from contextlib import ExitStack

    with TileContext(nc) as tc, ExitStack() as es:
        def sb(name, shape):
            return es.enter_context(nc.sbuf_tensor(name, shape, F32))

        cf_t = sb("cf_t", [U, NCO])
        cfn_t = sb("cfn_t", [U, NCO])
        wp1a_t = sb("wp1a_t", [U, U])
        wp1b_t = sb("wp1b_t", [U, U])
        bp1_t = sb("bp1_t", [U, 1])
        w2r_t = sb("w2r_t", [U, U])
        w2i_t = sb("w2i_t", [U, U])
        b2r_t = sb("b2r_t", [U, 1])
        b2i_t = sb("b2i_t", [U, 1])
        wp2a_t = sb("wp2a_t", [U, U])
        wp2b_t = sb("wp2b_t", [U, U])
        bp2_t = sb("bp2_t", [U, 1])
        wpr_t = sb("wpr_t", [U, PD])
        bpr_t = sb("bpr_t", [PD, 1])
        wh_t = sb("wh_t", [PD, NC_OUT])
        bh_t = sb("bh_t", [NC_OUT, 1])
        pool = es.enter_context(tc.tile_pool(name="work", bufs=3))
        psum_pool = es.enter_context(
            tc.tile_pool(name="psum", bufs=2, space="PSUM")
        )
        # NOTE: allocated AFTER the pools — a pool created as the last
        # allocation in scope mis-places its arena over earlier tensors
        # (verified empirically; the baseline's zst-after-pool has the same
        # effect).
        wA = sb("wA", [U, CW])
        wB = sb("wB", [U, CW])
        gring = sb("gring", [U, CHUNK * 2 * 2 * W])  # (slot, c, l, b)
        oring = sb("oring", [NC_OUT, CHUNK * W])

        for dst, src in (
            (cf_t, cf), (cfn_t, cfn), (wp1a_t, wp1a), (wp1b_t, wp1b), (bp1_t, bp1),
            (w2r_t, w2r), (w2i_t, w2i), (b2r_t, b2r), (b2i_t, b2i),
            (wp2a_t, wp2a), (wp2b_t, wp2b), (bp2_t, bp2), (wpr_t, wpr),
            (bpr_t, bpr), (wh_t, wh), (bh_t, bh), (wA, winit),
        ):
            nc.sync.dma_start(out=dst[:], in_=src[:])
        # warm matmul weights through DVE so PE operand deps collapse to
        # one semaphore (baseline-proven pattern for this walrus build)
        for wt in (wp1a_t, wp1b_t, w2r_t, w2i_t, wp2a_t, wp2b_t, wpr_t, wh_t):
            nc.vector.tensor_scalar_mul(out=wt[:], in0=wt[:], scalar1=1.0)
        nc.vector.memset(gring[:], 0.0)
        # prologue: g1 for tokens 0,1 into ring slots 0,1 (l=0 sub-cols)
        grv = gring[:].rearrange(
            "p (s c l b) -> p s c l b", s=CHUNK, c=2, l=2, b=W
        )
        nc.sync.dma_start(
            out=grv[:, :, :, 0, :],
            in_=g1[:, 0:CHUNK * 2 * W].rearrange(
                "p (s c b) -> p s c b", s=CHUNK, c=2, b=W
            ),
        )

        def cfs(f, k):
            """Scalar AP for chain coefficient k (0..DEG) of function f."""
            c = f * (DEG + 1) + k
            return cf_t[:, c:c + 1]

        def cfs_neg(f, k):
            """Negated-coefficient copy (second half of cfn rows)."""
            c = f * (DEG + 1) + k
            return cfn_t[:, c:c + 1]

        def emit_wave(j, i, win, wout, drain=False):
            """One token wave: map on DVE; g2 + head on PE/ACT.
            j: slot (static); i: For_i symbol or int; drain: skip g2."""
            s2 = (j + 2) % CHUNK
            gr = gring[:, j * 2 * LW:j * 2 * LW + LW]
            gi = gring[:, j * 2 * LW + LW:(j + 1) * 2 * LW]
            wr = win[:, 0:LW]
            wi = win[:, LW:CW]

            S = pool.tile([U, CW], F32, tag="S")
            u_ = pool.tile([U, LW], F32, tag="u")
            xi = pool.tile([U, LW], F32, tag="xi")
            if S_ON_ACT:
                nc.scalar.activation(out=S[:], in_=win[:], func=AF.Square)
            else:
                nc.vector.tensor_tensor(
                    out=S[:], in0=win[:], in1=win[:], op=OP.mult
                )
            if U_ON_POOL:
                nc.gpsimd.tensor_tensor(
                    out=u_[:], in0=S[:, 0:LW], in1=S[:, LW:CW], op=OP.add
                )
            else:
                nc.vector.tensor_tensor(
                    out=u_[:], in0=S[:, 0:LW], in1=S[:, LW:CW], op=OP.add
                )
            if XI_ON_ACT:
                nc.scalar.activation(
                    out=xi[:], in_=u_[:], func=AF.Identity,
                    bias=float(XI_B), scale=float(XI_A),
                )
            else:
                nc.vector.tensor_scalar(
                    out=xi[:], in0=u_[:], scalar1=float(XI_A), scalar2=float(XI_B),
                    op0=OP.mult, op1=OP.add,
                )
            # polynomial chains, interleaved; some chains on GPSIMD
            accs = [pool.tile([U, LW], F32, tag=f"acc{f}", name=f"acc{f}") for f in range(NF)]
            def eng(f):
                return nc.gpsimd if f < POOL_HORNER else nc.vector
            for f in range(NF):
                if STARTS_ON_ACT:
                    nc.scalar.activation(
                        out=accs[f][:], in_=xi[:], func=AF.Identity,
                        bias=0.0, scale=cfs(f, 0),
                    )
                else:
                    nc.vector.tensor_scalar(
                        out=accs[f][:], in0=xi[:], scalar1=cfs(f, 0),
                        scalar2=None, op0=OP.mult,
                    )
            for k in range(1, DEG):
                for f in range(NF):
                    eng(f).scalar_tensor_tensor(
                        out=accs[f][:], in0=accs[f][:], scalar=cfs(f, k),
                        in1=xi[:], op0=OP.add, op1=OP.mult,
                    )
            # c0 adds; also negated copies of F1i/F2i so every combine
            # product is a plain tensor_tensor (GPSIMD cannot run STT)
            F1i_n = pool.tile([U, LW], F32, tag="f1in", name="f1in")
            F2i_n = pool.tile([U, LW], F32, tag="f2in", name="f2in")
            nc.vector.tensor_scalar(
                out=F1i_n[:], in0=accs[1][:], scalar1=-1.0,
                scalar2=cfs_neg(1, DEG), op0=OP.mult, op1=OP.add,
            )
            nc.vector.tensor_scalar(
                out=F2i_n[:], in0=accs[3][:], scalar1=-1.0,
                scalar2=cfs_neg(3, DEG), op0=OP.mult, op1=OP.add,
            )
            for f in range(NF):
                if C0_ON_ACT:
                    nc.scalar.activation(
                        out=accs[f][:], in_=accs[f][:], func=AF.Identity,
                        bias=cfs(f, DEG), scale=1.0,
                    )
                else:
                    nc.vector.tensor_scalar(
                        out=accs[f][:], in0=accs[f][:], scalar1=cfs(f, DEG),
                        scalar2=None, op0=OP.add,
                    )
            F1r, F1i, F2r, F2i = accs[0], accs[1], accs[2], accs[3]
            # signed products into group tile G [64, (t:4)(c:2)(m:LW)], then
            # one X-axis reduce over t -> wout[:, (c m)]
            G = pool.tile([U, 4 * 2 * LW], F32, tag="G", name="G")
            prods = [
                (0, 0, F1r, wr), (1, 0, F1i_n, wi),
                (2, 0, F2r, gr), (3, 0, F2i_n, gi),
                (0, 1, F1r, wi), (1, 1, F1i, wr),
                (2, 1, F2r, gi), (3, 1, F2i, gr),
            ]
            for n_, (t, cmp_, Fk, vk) in enumerate(prods):
                off = (t * 2 + cmp_) * LW
                e = nc.gpsimd if n_ < POOL_PRODUCTS else nc.vector
                e.tensor_tensor(
                    out=G[:, off:off + LW], in0=Fk[:], in1=vk, op=OP.mult
                )
            Gv = G[:].rearrange("p (t c m) -> p c m t", t=4, c=2, m=LW)
            nc.vector.tensor_reduce(
                out=wout[:], in_=Gv, axis=mybir.AxisListType.X, op=OP.add
            )

            # ---- g2 pipeline (PE/ACT), from layer-1 cols of wout ----
            if not drain:
                ph1 = psum_pool.tile([U, W], F32, tag="m64")
                h1 = pool.tile([U, W], F32, tag="h1s")
                nc.tensor.matmul(
                    out=ph1[:], lhsT=wp1a_t[:], rhs=wout[:, 0:W],
                    start=True, stop=False,
                )
                nc.tensor.matmul(
                    out=ph1[:], lhsT=wp1b_t[:], rhs=wout[:, LW:LW + W],
                    start=False, stop=True,
                )
                nc.scalar.activation(
                    out=h1[:], in_=ph1[:], func=AF.Relu, bias=bp1_t[:, :]
                )
                pg = psum_pool.tile([U, W], F32, tag="m64")
                pgi = psum_pool.tile([U, W], F32, tag="m64")
                nc.tensor.matmul(
                    out=pg[:], lhsT=w2r_t[:], rhs=h1[:], start=True, stop=True
                )
                nc.scalar.activation(
                    out=grv[:, s2, 0, 1, :], in_=pg[:], func=AF.Relu,
                    bias=b2r_t[:, :],
                )
                nc.tensor.matmul(
                    out=pgi[:], lhsT=w2i_t[:], rhs=h1[:], start=True, stop=True
                )
                nc.scalar.activation(
                    out=grv[:, s2, 1, 1, :], in_=pgi[:], func=AF.Relu,
                    bias=b2i_t[:, :],
                )

            # ---- head (PE/ACT), from layer-2 cols of wout ----
            ph2 = psum_pool.tile([U, W], F32, tag="m64")
            h2 = pool.tile([U, W], F32, tag="h2s")
            nc.tensor.matmul(
                out=ph2[:], lhsT=wp2a_t[:], rhs=wout[:, W:LW],
                start=True, stop=False,
            )
            nc.tensor.matmul(
                out=ph2[:], lhsT=wp2b_t[:], rhs=wout[:, LW + W:CW],
                start=False, stop=True,
            )
            nc.scalar.activation(
                out=h2[:], in_=ph2[:], func=AF.Relu, bias=bp2_t[:, :]
            )
            ph3 = psum_pool.tile([PD, W], F32, tag="m20")
            h3 = pool.tile([PD, W], F32, tag="h3s")
            nc.tensor.matmul(
                out=ph3[:], lhsT=wpr_t[:], rhs=h2[:], start=True, stop=True
            )
            nc.scalar.activation(
                out=h3[:], in_=ph3[:], func=AF.Tanh, bias=bpr_t[:, :]
            )
            plg = psum_pool.tile([NC_OUT, W], F32, tag="m2")
            nc.tensor.matmul(
                out=plg[:], lhsT=wh_t[:], rhs=h3[:], start=True, stop=True
            )
            nc.scalar.activation(
                out=oring[:, j * W:(j + 1) * W], in_=plg[:],
                func=AF.Identity, bias=bh_t[:, :],
            )

        H = CHUNK // 2
        from contextlib import nullcontext
        outer = tc.For_i(0, repeat, 1, name="rep") if repeat > 1 else nullcontext()
        with outer:
            with tc.For_i(0, T // CHUNK, 1, name="tok") as i:
                last = True
                for j in range(CHUNK):
                    win, wout = (wA, wB) if j % 2 == 0 else (wB, wA)
                    emit_wave(j, i, win, wout)
                    # batched g1 prefetch for the next body, half-ring at a
                    # time once this body's reads of those slots are done
                    if last and j == H - 1:
                        nc.sync.dma_start(
                            out=grv[:, 0:H, :, 0, :],
                            in_=g1[:, bass.ds(
                                i * (CHUNK * 2 * W) + CHUNK * 2 * W,
                                H * 2 * W,
                            )].rearrange("p (s c b) -> p s c b", s=H, c=2, b=W),
                        )
                    if last and j == CHUNK - 1:
                        nc.sync.dma_start(
                            out=grv[:, H:CHUNK, :, 0, :],
                            in_=g1[:, bass.ds(
                                i * (CHUNK * 2 * W) + (CHUNK + H) * 2 * W,
                                H * 2 * W,
                            )].rearrange("p (s c b) -> p s c b", s=H, c=2, b=W),
                        )
                nc.sync.dma_start(
                    out=out[:, bass.ds(i * (CHUNK * W), CHUNK * W)], in_=oring[:]
                )
            # drain: 2 waves for layer-2 tokens T-2, T-1 (slots 0,1)
            for j in range(2):
                win, wout = (wA, wB) if j % 2 == 0 else (wB, wA)
                emit_wave(j, 0, win, wout, drain=True)
            nc.sync.dma_start(
                out=out[:, T * W:(T + 2) * W], in_=oring[:, 0:2 * W]
            )
            if repeat > 1:
                # re-load the first G ring + layer inits for the next pass
                nc.sync.dma_start(out=wA[:], in_=winit[:])
                nc.sync.dma_start(
                    out=grv[:, :, :, 0, :],
                    in_=g1[:, 0:CHUNK * 2 * W].rearrange(
                        "p (s c b) -> p s c b", s=CHUNK, c=2, b=W
                    ),
                )

    _split_waits(nc)
    _CACHE[key] = nc
    return nc


# ------------------------------------------------------------------- host
def _host_precompute(inp):
    f32 = np.float32
    E = inp["E"]
    t1r = (GS * np.maximum(E @ inp["W1r"] + inp["b1r"], 0)).astype(f32)
    t1i = (GS * np.maximum(E @ inp["W1i"] + inp["b1i"], 0)).astype(f32)
    beta = inp["om1"].astype(np.float64)
    assert np.allclose(inp["om1"], inp["om2"]), "kernel assumes om1 == om2"

    fk = ("cf", beta.tobytes())
    if fk not in _CACHE:
        _CACHE[fk] = _fit_coeffs(beta)
    cf, wpre = _CACHE[fk]

    winit = np.zeros((U, CW), f32)
    winit[:, 0:W] = 0.1 * SQDT  # l1 real
    winit[:, W:LW] = wpre.real[:, None]  # l2 real
    winit[:, LW + W:CW] = wpre.imag[:, None]  # l2 imag

    wp1s = (inp["Wp1"] / SQDT).astype(f32)
    wp2s = (inp["Wp2"] / SQDT).astype(f32)
    return {
        "cf": np.ascontiguousarray(cf.reshape(U, -1)),
        "cfn": np.ascontiguousarray(-cf.reshape(U, -1)),
        "winit": winit,
        "wp1a": np.ascontiguousarray(wp1s[:U]),
        "wp1b": np.ascontiguousarray(wp1s[U:]),
        "bp1": inp["bp1"][:, None].astype(f32),
        "w2r": (GS * inp["W2r"]).astype(f32),
        "w2i": (GS * inp["W2i"]).astype(f32),
        "b2r": (GS * inp["b2r"])[:, None].astype(f32),
        "b2i": (GS * inp["b2i"])[:, None].astype(f32),
        "wp2a": np.ascontiguousarray(wp2s[:U]),
        "wp2b": np.ascontiguousarray(wp2s[U:]),
        "bp2": inp["bp2"][:, None].astype(f32),
        "wpr": inp["Wpr"].astype(f32),
        "bpr": inp["bpr"][:, None].astype(f32),
        "wh": inp["Wh"].astype(f32),
        "bh": inp["bh"][:, None].astype(f32),
    }, t1r, t1i


def kernel(trace=False, repeat=1, **inputs):
    x = np.asarray(inputs["x"]).astype(np.int32)
    inp = {k: np.asarray(v).astype(np.float32) for k, v in inputs.items() if k != "x"}
    common, t1r, t1i = _host_precompute(inp)
    nc = _build(repeat=repeat)
    in_maps = []
    for c in range(N_CORES):
        xs = x[c * W:(c + 1) * W]  # [W, T]
        m = dict(common)
        # g1[u, (t, c, b)] = t1c[x[b, t], u]; +2 zero-pad tokens
        g = np.stack([t1r[xs], t1i[xs]], axis=0)  # [c, W, T, U]
        gp = np.zeros((U, T + CHUNK, 2, W), np.float32)
        gp[:, :T] = g.transpose(3, 2, 0, 1)
        m["g1"] = np.ascontiguousarray(gp.reshape(U, -1))
        in_maps.append(m)
    res = run_bass_kernel_spmd(
        nc, in_maps, core_ids=list(range(N_CORES)), trace=trace
    )
    out = np.empty((B, T, NC_OUT), np.float32)
    for c in range(N_CORES):
        o = res.results[c]["out"].reshape(NC_OUT, T + 2, W)
        out[c * W:(c + 1) * W] = o[:, 2:].transpose(2, 1, 0)
    kernel.last_result = res
    return out
